# revision 18
# baseline (speedup 1.0000x reference)
"""Trainium2 Bass kernel for the 3-layer ConvLSTM + dense head model.

Sharding: data-parallel over batch (16/8 = 2 per core) for the ConvLSTM
stack; K-sharded Dense1 (173056 contraction split 8 ways) + on-device
AllReduce + replicated Dense2/3 for the head.

All matmuls run in float32r (full-rate fp32-precision mode on TRN2).
Conv layers are computed as tap-accumulated matmuls in PSUM:
  - hidden (SAME) convs read shifted views of a zero-padded SBUF h state
  - input (VALID) convs are fused into the same PSUM accumulation group
  - layer-1's input conv uses host-side im2col (K=125)
  - 64-channel contractions pack 2 taps per matmul via a partition-
    duplicated, x-shifted copy (K=64 -> 128)
"""
import os
import sys
import types
from contextlib import ExitStack

import numpy as np

import concourse.bacc as bacc
import concourse.bass as bass
import concourse.mybir as mybir
import concourse.tile as tile
from concourse.bass_utils import run_bass_kernel_spmd

F32 = mybir.dt.float32
F32R = mybir.dt.float32r
F16 = mybir.dt.float16
AF = mybir.ActivationFunctionType
ALU = mybir.AluOpType

NCORES = 8
BC = 2          # batch per core
T = 6

# layer geometry
L1 = dict(Ho=60, Wo=60, F=128, NCT=4, Wp=64, Cin=128)
L2 = dict(Ho=56, Wo=56, F=64, NCT=2, Wp=60, Cin=128)
L3 = dict(Ho=52, Wo=52, F=64, NCT=2, Wp=56, Cin=64)

LAST_EXEC_NS = []
LAST_RESULTS = []

_CACHE = {}


def _want_trace():
    if os.environ.get("BASS_KERNEL_TRACE") != "1":
        return False
    try:
        _install_ntff_hook()
        return True
    except Exception:
        return False


def _install_ntff_hook():
    if "antenv.axon_hooks" in sys.modules:
        return
    mod = types.ModuleType("antenv.axon_hooks")
    mod._hook = None
    mod.set_axon_ntff_profile_hook = lambda h: setattr(mod, "_hook", h)
    mod.get_axon_ntff_profile_hook = lambda: mod._hook
    sys.modules["antenv.axon_hooks"] = mod
    import antenv
    antenv.axon_hooks = mod
    from trn_agent_boot.trn_boot import _ntff_profile_via_ctypes
    hook = _ntff_profile_via_ctypes("/opt/axon/libaxon_pjrt.so")
    if hook is not None:
        mod.set_axon_ntff_profile_hook(hook)


def _tap_view(t_ap, off, nrow, wp, wo):
    """[128, nrow, wo] strided view at free-dim offset `off`, row stride wp."""
    return t_ap[:, off:off + nrow * wp].rearrange("p (r w) -> p r w", r=nrow)[:, :, :wo]


def _blocks(ho, nrow):
    """Split ho rows into blocks of nrow (last blocks adjusted to keep N>=256)."""
    if ho == 60:                # L1: 6x8 + 2x6 (min N = 6*60=360)
        return [(i * 8, 8) for i in range(6)] + [(48, 6), (54, 6)]
    if ho == 56:                # L2: 7x8 (N = 448)
        return [(i * 8, 8) for i in range(7)]
    if ho == 52:                # L3: 5x9 + 1x7 (N = 468 / 364)
        return [(i * 9, 9) for i in range(5)] + [(45, 7)]
    raise ValueError(ho)


def _build_launch_a(debug=False):
    nc = bacc.Bacc("TRN2", target_bir_lowering=False, debug=False,
                   num_devices=NCORES)

    xim = nc.dram_tensor("xim", [BC * T, 125, 3600], F16, kind="ExternalInput").ap()
    wx1 = nc.dram_tensor("wx1", [125, 512], F16, kind="ExternalInput").ap()
    wh1 = nc.dram_tensor("wh1", [128, 25 * 512], F16, kind="ExternalInput").ap()
    wx2 = nc.dram_tensor("wx2", [128, 25 * 256], F16, kind="ExternalInput").ap()
    wh2p = nc.dram_tensor("wh2p", [128, 15 * 256], F16, kind="ExternalInput").ap()
    wx3p = nc.dram_tensor("wx3p", [128, 15 * 256], F16, kind="ExternalInput").ap()
    wh3p = nc.dram_tensor("wh3p", [128, 15 * 256], F16, kind="ExternalInput").ap()
    b1v = nc.dram_tensor("b1v", [128, 4], F32, kind="ExternalInput").ap()
    b2v = nc.dram_tensor("b2v", [64, 4], F32, kind="ExternalInput").ap()
    b3v = nc.dram_tensor("b3v", [64, 4], F32, kind="ExternalInput").ap()
    h3o = nc.dram_tensor("h3o", [128, 2704], F16, kind="ExternalOutput").ap()
    if debug:
        h1dbg = nc.dram_tensor("h1dbg", [BC * T, 128, 3600], F16, kind="ExternalOutput").ap()
        h2dbg = nc.dram_tensor("h2dbg", [BC * T, 64, 3136], F16, kind="ExternalOutput").ap()

    with TileCtx(nc) as tc, ExitStack() as top:
        dram = top.enter_context(tc.tile_pool(name="dram", bufs=1, space="DRAM"))
        if debug:
            h1seq, h2seq = h1dbg, h2dbg
        else:
            h1seq = dram.tile([BC * T, 128, 3600], F16)
            h2seq = dram.tile([BC * T, 64, 3136], F16)

        # ---------------- phase 1: ConvLSTM(5 -> 128), 60x60 ----------------
        with ExitStack() as ctx:
            wpool = ctx.enter_context(tc.tile_pool(name="w1", bufs=1))
            spool = ctx.enter_context(tc.tile_pool(name="s1", bufs=1))
            ipool = ctx.enter_context(tc.tile_pool(name="i1", bufs=6))
            gpool = ctx.enter_context(tc.tile_pool(name="g1", bufs=2))
            ppool = ctx.enter_context(tc.tile_pool(name="p1", bufs=8, space="PSUM"))

            wx1_t = wpool.tile([125, 512], F16, tag="wx1")
            wh1_t = wpool.tile([128, 25 * 512], F16, tag="wh1")
            nc.gpsimd.dma_start(out=wx1_t[:, :], in_=wx1[:, :])
            nc.gpsimd.dma_start(out=wh1_t[:, :], in_=wh1[:, :])
            bsb = wpool.tile([128, 4], F32, tag="b1raw")
            bsig = wpool.tile([128, 4], F32, tag="b1sig")
            nc.sync.dma_start(out=bsb[:, :], in_=b1v[:, :])
            nc.vector.tensor_scalar(bsig[:, :], bsb[:, :], 0.2, 0.5, ALU.mult, ALU.add)

            hpad = [spool.tile([128, 4160], F16, tag=f"hpad{b}", name=f"hpad1_{b}") for b in range(BC)]
            hcur = [spool.tile([128, 3600], F16, tag=f"hcur{b}", name=f"hcur1_{b}") for b in range(BC)]
            cst = [spool.tile([128, 3600], F32, tag=f"c1_{b}", name=f"c1_{b}") for b in range(BC)]
            for b in range(BC):
                nc.gpsimd.memset(hpad[b][:, :].bitcast(F32), 0.0)

            wp, wo = L1["Wp"], L1["Wo"]
            for t in range(T):
                for b in range(BC):
                    cts = (0, 2, 3) if t == 0 else (0, 1, 2, 3)
                    for y0, nrow in _blocks(60, 8):
                        n = nrow * wo
                        xb = ipool.tile([125, 480], F16, tag="xim")
                        nc.sync.dma_start(out=xb[:, :n],
                                          in_=xim[b * T + t, :, y0 * 60:y0 * 60 + n])
                        ps = {}
                        for ct in cts:
                            acc = ppool.tile([128, 480], F32, tag="ps")
                            ps[ct] = acc
                            nc.tensor.matmul(
                                acc[:, :n], wx1_t[:, ct * 128:(ct + 1) * 128],
                                xb[:, :n],
                                start=True, stop=(t == 0))
                            if t > 0:
                                for tap in range(25):
                                    dy, dx = divmod(tap, 5)
                                    rhs = _tap_view(hpad[b], (y0 + dy) * wp + dx, nrow, wp, wo)
                                    nc.tensor.matmul(
                                        acc[:, :n],
                                        wh1_t[:, tap * 512 + ct * 128:tap * 512 + (ct + 1) * 128],
                                        rhs, start=False, stop=(tap == 24))
                        sl = slice(y0 * wo, y0 * wo + n)
                        g = {}
                        for ct in cts:
                            gt = gpool.tile([128, 480], F32, tag=f"g{ct}")
                            g[ct] = gt
                            if ct == 2:
                                nc.scalar.activation(gt[:, :n], ps[ct][:, :n], AF.Tanh,
                                                     bias=bsb[:, 2:3])
                            else:
                                nc.scalar.activation(gt[:, :n], ps[ct][:, :n], AF.Identity,
                                                     bias=bsig[:, ct:ct + 1], scale=0.2)
                                nc.vector.tensor_scalar(gt[:, :n], gt[:, :n], 0.0, 1.0,
                                                        ALU.max, ALU.min)
                        if t == 0:
                            nc.vector.tensor_mul(cst[b][:, sl], g[0][:, :n], g[2][:, :n])
                        else:
                            t1 = gpool.tile([128, 480], F32, tag="t1")
                            t2 = gpool.tile([128, 480], F32, tag="t2")
                            nc.vector.tensor_mul(t1[:, :n], g[1][:, :n], cst[b][:, sl])
                            nc.vector.tensor_mul(t2[:, :n], g[0][:, :n], g[2][:, :n])
                            nc.vector.tensor_add(cst[b][:, sl], t1[:, :n], t2[:, :n])
                        tc_t = gpool.tile([128, 480], F32, tag="tct")
                        nc.scalar.activation(tc_t[:, :n], cst[b][:, sl], AF.Tanh)
                        nc.vector.tensor_mul(hcur[b][:, sl], g[3][:, :n], tc_t[:, :n])
                    # end blocks: update padded state + spill sequence
                    dst = _tap_view(hpad[b], 2 * wp + 2, wo, wp, wo)
                    src = hcur[b][:, :].rearrange("p (r w) -> p r w", r=wo)
                    nc.vector.tensor_copy(dst, src)
                    nc.sync.dma_start(out=h1seq[b * T + t, :, :], in_=hcur[b][:, :])

        # ---------------- phase 2: ConvLSTM(128 -> 64), 56x56 ----------------
        with ExitStack() as ctx:
            wpool = ctx.enter_context(tc.tile_pool(name="w2", bufs=1))
            spool = ctx.enter_context(tc.tile_pool(name="s2", bufs=1))
            ipool = ctx.enter_context(tc.tile_pool(name="i2", bufs=2))
            gpool = ctx.enter_context(tc.tile_pool(name="g2", bufs=3))
            ppool = ctx.enter_context(tc.tile_pool(name="p2", bufs=8, space="PSUM"))

            wx2_t = wpool.tile([128, 25 * 256], F16, tag="wx2")
            wh2_t = wpool.tile([128, 15 * 256], F16, tag="wh2")
            nc.gpsimd.dma_start(out=wx2_t[:, :], in_=wx2[:, :])
            nc.gpsimd.dma_start(out=wh2_t[:, :], in_=wh2p[:, :])
            bsb = wpool.tile([64, 4], F32, tag="b2raw")
            bsig = wpool.tile([64, 4], F32, tag="b2sig")
            nc.sync.dma_start(out=bsb[:, :], in_=b2v[:, :])
            nc.vector.tensor_scalar(bsig[:, :], bsb[:, :], 0.2, 0.5, ALU.mult, ALU.add)

            hpad = [spool.tile([128, 3664], F16, tag=f"hpad{b}", name=f"hpad2_{b}") for b in range(BC)]
            hcur = [spool.tile([64, 3136], F16, tag=f"hcur{b}", name=f"hcur2_{b}") for b in range(BC)]
            cst = [spool.tile([64, 3136], F32, tag=f"c2_{b}", name=f"c2_{b}") for b in range(BC)]
            for b in range(BC):
                nc.gpsimd.memset(hpad[b][:, :].bitcast(F32), 0.0)

            wp, wo = L2["Wp"], L2["Wo"]
            for t in range(T):
                for b in range(BC):
                    img = ipool.tile([128, 3640], F16, tag="h1in")
                    nc.sync.dma_start(out=img[:, :3600], in_=h1seq[b * T + t, :, :])
                    for y0, nrow in _blocks(56, 8):
                        n = nrow * wo
                        ps = []
                        for ct in range(2):
                            acc = ppool.tile([128, 448], F32, tag="ps")
                            ps.append(acc)
                            first = True
                            for tap in range(25):
                                dy, dx = divmod(tap, 5)
                                rhs = _tap_view(img, (y0 + dy) * 60 + dx, nrow, 60, wo)
                                nc.tensor.matmul(
                                    acc[:, :n],
                                    wx2_t[:, tap * 256 + ct * 128:tap * 256 + (ct + 1) * 128],
                                    rhs, start=first,
                                    stop=(t == 0 and tap == 24))
                                first = False
                            if t > 0:
                                for e in range(15):
                                    dy, k = divmod(e, 3)
                                    rhs = _tap_view(hpad[b], (y0 + dy) * wp + 2 * k, nrow, wp, wo)
                                    nc.tensor.matmul(
                                        acc[:, :n],
                                        wh2_t[:, e * 256 + ct * 128:e * 256 + (ct + 1) * 128],
                                        rhs, start=False, stop=(e == 14))
                        sl = slice(y0 * wo, y0 * wo + n)
                        # gates: ps[0]=[i;f], ps[1]=[c;o]
                        si = gpool.tile([64, 448], F32, tag="si")
                        nc.scalar.activation(si[:, :n], ps[0][0:64, :n], AF.Identity,
                                             bias=bsig[:, 0:1], scale=0.2)
                        nc.vector.tensor_scalar(si[:, :n], si[:, :n], 0.0, 1.0,
                                                ALU.max, ALU.min)
                        gt = gpool.tile([64, 448], F32, tag="gt")
                        nc.scalar.activation(gt[:, :n], ps[1][0:64, :n], AF.Tanh,
                                             bias=bsb[:, 2:3])
                        so = gpool.tile([64, 448], F32, tag="so")
                        nc.scalar.activation(so[:, :n], ps[1][64:128, :n], AF.Identity,
                                             bias=bsig[:, 3:4], scale=0.2)
                        nc.vector.tensor_scalar(so[:, :n], so[:, :n], 0.0, 1.0,
                                                ALU.max, ALU.min)
                        if t == 0:
                            nc.vector.tensor_mul(cst[b][:, sl], si[:, :n], gt[:, :n])
                        else:
                            sf = gpool.tile([64, 448], F32, tag="sf")
                            nc.scalar.activation(sf[:, :n], ps[0][64:128, :n], AF.Identity,
                                                 bias=bsig[:, 1:2], scale=0.2)
                            nc.vector.tensor_scalar(sf[:, :n], sf[:, :n], 0.0, 1.0,
                                                    ALU.max, ALU.min)
                            t1 = gpool.tile([64, 448], F32, tag="t1")
                            t2 = gpool.tile([64, 448], F32, tag="t2")
                            nc.vector.tensor_mul(t1[:, :n], sf[:, :n], cst[b][:, sl])
                            nc.vector.tensor_mul(t2[:, :n], si[:, :n], gt[:, :n])
                            nc.vector.tensor_add(cst[b][:, sl], t1[:, :n], t2[:, :n])
                        tc_t = gpool.tile([64, 448], F32, tag="tct")
                        nc.scalar.activation(tc_t[:, :n], cst[b][:, sl], AF.Tanh)
                        nc.vector.tensor_mul(hcur[b][:, sl], so[:, :n], tc_t[:, :n])
                    # end blocks: padded dup state (rows 0:64 plain, 64:128 x-shifted)
                    src = hcur[b][:, :].rearrange("p (r w) -> p r w", r=wo)
                    dst0 = hpad[b][0:64, 2 * wp + 2:2 * wp + 2 + wo * wp] \
                        .rearrange("p (r w) -> p r w", r=wo)[:, :, :wo]
                    nc.vector.tensor_copy(dst0, src)
                    dst1 = hpad[b][64:128, 2 * wp + 1:2 * wp + 1 + wo * wp] \
                        .rearrange("p (r w) -> p r w", r=wo)[:, :, :wo]
                    nc.vector.tensor_copy(dst1, src)
                    nc.sync.dma_start(out=h2seq[b * T + t, :, :], in_=hcur[b][:, :])

        # ---------------- phase 3: ConvLSTM(64 -> 64), 52x52 ----------------
        with ExitStack() as ctx:
            wpool = ctx.enter_context(tc.tile_pool(name="w3", bufs=1))
            spool = ctx.enter_context(tc.tile_pool(name="s3", bufs=1))
            ipool = ctx.enter_context(tc.tile_pool(name="i3", bufs=2))
            gpool = ctx.enter_context(tc.tile_pool(name="g3", bufs=3))
            ppool = ctx.enter_context(tc.tile_pool(name="p3", bufs=8, space="PSUM"))

            wx3_t = wpool.tile([128, 15 * 256], F16, tag="wx3")
            wh3_t = wpool.tile([128, 15 * 256], F16, tag="wh3")
            nc.gpsimd.dma_start(out=wx3_t[:, :], in_=wx3p[:, :])
            nc.gpsimd.dma_start(out=wh3_t[:, :], in_=wh3p[:, :])
            bsb = wpool.tile([64, 4], F32, tag="b3raw")
            bsig = wpool.tile([64, 4], F32, tag="b3sig")
            nc.sync.dma_start(out=bsb[:, :], in_=b3v[:, :])
            nc.vector.tensor_scalar(bsig[:, :], bsb[:, :], 0.2, 0.5, ALU.mult, ALU.add)

            hpad = [spool.tile([128, 3300], F16, tag=f"hpad{b}", name=f"hpad3_{b}") for b in range(BC)]
            hcur = [spool.tile([64, 2704], F16, tag=f"hcur{b}", name=f"hcur3_{b}") for b in range(BC)]
            cst = [spool.tile([64, 2704], F32, tag=f"c3_{b}", name=f"c3_{b}") for b in range(BC)]
            for b in range(BC):
                nc.gpsimd.memset(hpad[b][:, :].bitcast(F32), 0.0)

            wp, wo = L3["Wp"], L3["Wo"]
            for t in range(T):
                for b in range(BC):
                    # build dup input [128, 3196]: rows 0:64 = h2 image,
                    # rows 64:128 = x-shifted by 1 (valid 56-wide coords)
                    img = ipool.tile([128, 3196], F16, tag="h2in")
                    nc.sync.dma_start(out=img[0:64, :3136], in_=h2seq[b * T + t, :, :])
                    nc.vector.tensor_copy(img[64:128, 0:3135], img[0:64, 1:3136])
                    for y0, nrow in _blocks(52, 9):
                        n = nrow * wo
                        ps = []
                        for ct in range(2):
                            acc = ppool.tile([128, 468], F32, tag="ps")
                            ps.append(acc)
                            first = True
                            for e in range(15):
                                dy, k = divmod(e, 3)
                                rhs = _tap_view(img, (y0 + dy) * 56 + 2 * k, nrow, 56, wo)
                                nc.tensor.matmul(
                                    acc[:, :n],
                                    wx3_t[:, e * 256 + ct * 128:e * 256 + (ct + 1) * 128],
                                    rhs, start=first,
                                    stop=(t == 0 and e == 14))
                                first = False
                            if t > 0:
                                for e in range(15):
                                    dy, k = divmod(e, 3)
                                    rhs = _tap_view(hpad[b], (y0 + dy) * wp + 2 * k, nrow, wp, wo)
                                    nc.tensor.matmul(
                                        acc[:, :n],
                                        wh3_t[:, e * 256 + ct * 128:e * 256 + (ct + 1) * 128],
                                        rhs, start=False, stop=(e == 14))
                        sl = slice(y0 * wo, y0 * wo + n)
                        si = gpool.tile([64, 468], F32, tag="si")
                        nc.scalar.activation(si[:, :n], ps[0][0:64, :n], AF.Identity,
                                             bias=bsig[:, 0:1], scale=0.2)
                        nc.vector.tensor_scalar(si[:, :n], si[:, :n], 0.0, 1.0,
                                                ALU.max, ALU.min)
                        gt = gpool.tile([64, 468], F32, tag="gt")
                        nc.scalar.activation(gt[:, :n], ps[1][0:64, :n], AF.Tanh,
                                             bias=bsb[:, 2:3])
                        so = gpool.tile([64, 468], F32, tag="so")
                        nc.scalar.activation(so[:, :n], ps[1][64:128, :n], AF.Identity,
                                             bias=bsig[:, 3:4], scale=0.2)
                        nc.vector.tensor_scalar(so[:, :n], so[:, :n], 0.0, 1.0,
                                                ALU.max, ALU.min)
                        if t == 0:
                            nc.vector.tensor_mul(cst[b][:, sl], si[:, :n], gt[:, :n])
                        else:
                            sf = gpool.tile([64, 468], F32, tag="sf")
                            nc.scalar.activation(sf[:, :n], ps[0][64:128, :n], AF.Identity,
                                                 bias=bsig[:, 1:2], scale=0.2)
                            nc.vector.tensor_scalar(sf[:, :n], sf[:, :n], 0.0, 1.0,
                                                    ALU.max, ALU.min)
                            t1 = gpool.tile([64, 468], F32, tag="t1")
                            t2 = gpool.tile([64, 468], F32, tag="t2")
                            nc.vector.tensor_mul(t1[:, :n], sf[:, :n], cst[b][:, sl])
                            nc.vector.tensor_mul(t2[:, :n], si[:, :n], gt[:, :n])
                            nc.vector.tensor_add(cst[b][:, sl], t1[:, :n], t2[:, :n])
                        tc_t = gpool.tile([64, 468], F32, tag="tct")
                        nc.scalar.activation(tc_t[:, :n], cst[b][:, sl], AF.Tanh)
                        nc.vector.tensor_mul(hcur[b][:, sl], so[:, :n], tc_t[:, :n])
                    if t < T - 1:
                        src = hcur[b][:, :].rearrange("p (r w) -> p r w", r=wo)
                        dst0 = hpad[b][0:64, 2 * wp + 2:2 * wp + 2 + wo * wp] \
                            .rearrange("p (r w) -> p r w", r=wo)[:, :, :wo]
                        nc.vector.tensor_copy(dst0, src)
                        dst1 = hpad[b][64:128, 2 * wp + 1:2 * wp + 1 + wo * wp] \
                            .rearrange("p (r w) -> p r w", r=wo)[:, :, :wo]
                        nc.vector.tensor_copy(dst1, src)
                    else:
                        nc.sync.dma_start(out=h3o[b * 64:(b + 1) * 64, :], in_=hcur[b][:, :])

    nc.compile()
    return nc


def TileCtx(nc):
    return tile.TileContext(nc, pool_alloc_mode="queue")


def _build_launch_b():
    KS = 173056 // NCORES      # 21632 contraction rows per core
    KT = KS // 128             # 169 k-tiles
    CH = 16                    # k-tiles per DMA chunk
    nc = bacc.Bacc("TRN2", target_bir_lowering=False, debug=False,
                   num_devices=NCORES)
    ztk = nc.dram_tensor("ztk", [128, (KS // 128) * 16], F16, kind="ExternalInput").ap()
    # wd1 halves pre-shuffled on host to SBUF layout [p, kt*512] for flat DMA
    wd1a = nc.dram_tensor("wd1a", [128, KT * 512], F16, kind="ExternalInput").ap()
    wd1b = nc.dram_tensor("wd1b", [128, KT * 512], F16, kind="ExternalInput").ap()
    wd2 = nc.dram_tensor("wd2", [128, 8 * 1024], F16, kind="ExternalInput").ap()
    wd3 = nc.dram_tensor("wd3", [128, 8 * 4], F16, kind="ExternalInput").ap()
    bd1 = nc.dram_tensor("bd1", [128, 8], F32, kind="ExternalInput").ap()
    bd2 = nc.dram_tensor("bd2", [128, 8], F32, kind="ExternalInput").ap()
    bd3 = nc.dram_tensor("bd3", [4, 1], F32, kind="ExternalInput").ap()
    eye = nc.dram_tensor("eye16", [16, 16], F32, kind="ExternalInput").ap()
    out = nc.dram_tensor("out", [4, 16], F32, kind="ExternalOutput").ap()

    with TileCtx(nc) as tc, ExitStack() as ctx:
        cpool = ctx.enter_context(tc.tile_pool(name="cst", bufs=1))
        wpool = ctx.enter_context(tc.tile_pool(name="wd1", bufs=3))
        apool = ctx.enter_context(tc.tile_pool(name="act", bufs=1))
        ppool = ctx.enter_context(tc.tile_pool(name="ps", bufs=1, space="PSUM"))
        dram = ctx.enter_context(tc.tile_pool(name="dram", bufs=1, space="DRAM"))

        # z^T slice: [KS,16] -> [128, KT*16]
        zt = cpool.tile([128, KT * 16], F16, tag="zt")
        nc.gpsimd.dma_start(out=zt[:, :], in_=ztk[:, :])
        eye_t = cpool.tile([16, 16], F32, tag="eye")
        nc.sync.dma_start(out=eye_t[:, :], in_=eye[:, :])
        b1t = cpool.tile([128, 8], F32, tag="b1")
        b2t = cpool.tile([128, 8], F32, tag="b2")
        b3t = cpool.tile([4, 1], F32, tag="b3")
        nc.sync.dma_start(out=b1t[:, :], in_=bd1[:, :])
        nc.sync.dma_start(out=b2t[:, :], in_=bd2[:, :])
        nc.sync.dma_start(out=b3t[:, :], in_=bd3[:, :])
        wd2_t = cpool.tile([128, 8 * 1024], F16, tag="wd2")
        nc.gpsimd.dma_start(out=wd2_t[:, :], in_=wd2[:, :])
        wd3_t = cpool.tile([128, 8 * 4], F16, tag="wd3")
        nc.gpsimd.dma_start(out=wd3_t[:, :], in_=wd3[:, :])

        # dense1 in two column halves; half 0's AllReduce overlaps half 1
        bin_ = [dram.tile([16, 512], F32, name=f"bin{h}") for h in range(2)]
        bout = [dram.tile([16, 512], F32, name=f"bout{h}") for h in range(2)]
        chunks = []
        _off = 0
        for _sz in [2, 2, 4]:
            chunks.append((_off, _sz))
            _off += _sz
        while _off < KT:
            _sz = min(CH, KT - _off)
            chunks.append((_off, _sz))
            _off += _sz
        for h, src in ((0, wd1a), (1, wd1b)):
            acc = ppool.tile([16, 512], F32, tag="acc", name=f"acc{h}", bufs=2)
            for c0, cn in chunks:
                w_t = wpool.tile([128, CH * 512], F16, tag="w", name=f"w{h}_{c0}")
                nc.sync.dma_start(out=w_t[:, :cn * 512],
                                  in_=src[:, c0 * 512:(c0 + cn) * 512])
                for i in range(cn):
                    kt = c0 + i
                    nc.tensor.matmul(acc[:, :], zt[:, kt * 16:(kt + 1) * 16],
                                     w_t[:, i * 512:(i + 1) * 512],
                                     start=(kt == 0), stop=(kt == KT - 1))
            a1p = apool.tile([16, 512], F32, tag="a1p", name=f"a1p{h}", bufs=2)
            nc.vector.tensor_copy(a1p[:, :], acc[:, :])
            nc.sync.dma_start(out=bin_[h][:, :], in_=a1p[:, :])
            nc.gpsimd.collective_compute(
                "AllReduce", ALU.add,
                replica_groups=[list(range(NCORES))],
                ins=[bin_[h][:].opt()], outs=[bout[h][:].opt()])
        a1f = apool.tile([16, 1024], F32, tag="a1f")
        for h in range(2):
            nc.sync.dma_start(out=a1f[:, h * 512:(h + 1) * 512], in_=bout[h][:, :])

        # transpose a1 -> [128,16] tiles; bias+relu; dense2
        a1t = apool.tile([128, 8 * 16], F16, tag="a1t")
        for ct in range(8):
            pt = ppool.tile([128, 16], F32, tag="pt", bufs=2)
            nc.tensor.transpose(pt[:, :], a1f[:, ct * 128:(ct + 1) * 128],
                                eye_t[:, :])
            nc.scalar.activation(a1t[:, ct * 16:(ct + 1) * 16], pt[:, :], AF.Relu,
                                 bias=b1t[:, ct:ct + 1])
        a2t = apool.tile([128, 8 * 16], F16, tag="a2t")
        for ct in range(8):
            p2 = ppool.tile([128, 16], F32, tag="p2", bufs=2)
            for kt in range(8):
                nc.tensor.matmul(
                    p2[:, :],
                    wd2_t[:, kt * 1024 + ct * 128:kt * 1024 + (ct + 1) * 128],
                    a1t[:, kt * 16:(kt + 1) * 16],
                    start=(kt == 0), stop=(kt == 7))
            nc.scalar.activation(a2t[:, ct * 16:(ct + 1) * 16], p2[:, :], AF.Relu,
                                 bias=b2t[:, ct:ct + 1])
        p3 = ppool.tile([4, 16], F32, tag="p3", bufs=1)
        for kt in range(8):
            nc.tensor.matmul(p3[:, :], wd3_t[:, kt * 4:(kt + 1) * 4],
                             a2t[:, kt * 16:(kt + 1) * 16],
                             start=(kt == 0), stop=(kt == 7))
        o_t = apool.tile([4, 16], F32, tag="o")
        nc.vector.tensor_scalar(o_t[:, :], p3[:, :], b3t[:, 0:1], None, ALU.add)
        nc.sync.dma_start(out=out[:, :], in_=o_t[:, :])

    nc.compile()
    return nc


def _pack_pairs(w):
    """(5,5,64,256) -> [128, 15*256]: pair taps (dy,2k)+(dy,2k+1) along K."""
    out = np.zeros((128, 15, 256), np.float32)
    for dy in range(5):
        for k in range(3):
            e = dy * 3 + k
            out[0:64, e] = w[dy, 2 * k]
            if 2 * k + 1 < 5:
                out[64:128, e] = w[dy, 2 * k + 1]
    return np.ascontiguousarray(out.reshape(128, 15 * 256)).astype(np.float16)


def _host_prep_a(x, Wx1, Wh1, b1, Wx2, Wh2, b2, Wx3, Wh3, b3):
    xw = np.lib.stride_tricks.sliding_window_view(x, (5, 5), axis=(2, 3))
    # [b,t,y,x,c,dy,dx] -> [b,t,(dy,dx,c),(y,x)]
    xim = np.ascontiguousarray(
        xw.transpose(0, 1, 5, 6, 4, 2, 3).reshape(16, 6, 125, 3600), np.float32)
    shared = dict(
        wx1=np.ascontiguousarray(Wx1.reshape(125, 512), np.float32).astype(np.float16),
        wh1=np.ascontiguousarray(
            Wh1.reshape(25, 128, 512).transpose(1, 0, 2).reshape(128, 25 * 512)).astype(np.float16),
        wx2=np.ascontiguousarray(
            Wx2.reshape(25, 128, 256).transpose(1, 0, 2).reshape(128, 25 * 256)).astype(np.float16),
        wh2p=_pack_pairs(Wh2.reshape(5, 5, 64, 256)),
        wx3p=_pack_pairs(Wx3.reshape(5, 5, 64, 256)),
        wh3p=_pack_pairs(Wh3.reshape(5, 5, 64, 256)),
        b1v=np.ascontiguousarray(b1.reshape(4, 128).T, np.float32),
        b2v=np.ascontiguousarray(b2.reshape(4, 64).T, np.float32),
        b3v=np.ascontiguousarray(b3.reshape(4, 64).T, np.float32),
    )
    in_maps = []
    for j in range(NCORES):
        m = dict(shared)
        m["xim"] = np.ascontiguousarray(
            xim[2 * j:2 * j + 2].reshape(12, 125, 3600)).astype(np.float16)
        in_maps.append(m)
    return in_maps


def _run(nc, in_maps, trace):
    res = run_bass_kernel_spmd(nc, in_maps, core_ids=list(range(NCORES)),
                               trace=trace)
    if res.exec_time_ns is not None:
        LAST_EXEC_NS.append(res.exec_time_ns)
    LAST_RESULTS.append(res)
    return res


def kernel(x, Wx1, Wh1, b1, Wx2, Wh2, b2, Wx3, Wh3, b3,
           Wd1, bd1, Wd2, bd2, Wd3, bd3):
    trace = _want_trace()
    LAST_EXEC_NS.clear()
    LAST_RESULTS.clear()
    x = np.asarray(x, np.float32)

    if "a" not in _CACHE:
        _CACHE["a"] = _build_launch_a()
    in_a = _host_prep_a(x, np.asarray(Wx1), np.asarray(Wh1), np.asarray(b1),
                        np.asarray(Wx2), np.asarray(Wh2), np.asarray(b2),
                        np.asarray(Wx3), np.asarray(Wh3), np.asarray(b3))
    res_a = _run(_CACHE["a"], in_a, trace)

    h3 = np.stack([res_a.results[j]["h3o"][(b % 2) * 64:(b % 2) * 64 + 64]
                   for b, j in [(b, b // 2) for b in range(16)]]).astype(np.float32)
    zt = np.ascontiguousarray(h3.transpose(2, 1, 0).reshape(173056, 16), np.float32)

    if "b" not in _CACHE:
        _CACHE["b"] = _build_launch_b()
    KS = 173056 // NCORES
    KT = KS // 128
    Wd1 = np.asarray(Wd1, np.float32)
    shared_b = dict(
        wd2=np.ascontiguousarray(
            np.asarray(Wd2, np.float32).reshape(8, 128, 1024)
            .transpose(1, 0, 2).reshape(128, 8 * 1024)).astype(np.float16),
        wd3=np.ascontiguousarray(
            np.asarray(Wd3, np.float32).reshape(8, 128, 4)
            .transpose(1, 0, 2).reshape(128, 32)).astype(np.float16),
        bd1=np.ascontiguousarray(np.asarray(bd1).reshape(8, 128).T, np.float32),
        bd2=np.ascontiguousarray(np.asarray(bd2).reshape(8, 128).T, np.float32),
        bd3=np.asarray(bd3, np.float32).reshape(4, 1),
        eye16=np.eye(16, dtype=np.float32),
    )
    in_b = []
    for j in range(NCORES):
        m = dict(shared_b)
        zs = zt[j * KS:(j + 1) * KS]                    # [KS, 16]
        m["ztk"] = np.ascontiguousarray(
            zs.reshape(KT, 128, 16).transpose(1, 0, 2).reshape(128, -1)).astype(np.float16)
        for key, h in (("wd1a", 0), ("wd1b", 1)):
            m[key] = np.ascontiguousarray(
                Wd1[j * KS:(j + 1) * KS, h * 512:(h + 1) * 512]
                .reshape(KT, 128, 512).transpose(1, 0, 2)
                .reshape(128, KT * 512)).astype(np.float16)
        in_b.append(m)
    res_b = _run(_CACHE["b"], in_b, trace)

    return np.ascontiguousarray(res_b.results[0]["out"].T, np.float32)



# revision 20
# speedup vs baseline: 1.0003x; 1.0003x over previous
"""Trainium2 Bass kernel for the 3-layer ConvLSTM + dense head model.

Sharding: data-parallel over batch (16/8 = 2 per core) for the ConvLSTM
stack; K-sharded Dense1 (173056 contraction split 8 ways) + on-device
AllReduce + replicated Dense2/3 for the head.

All matmul operands are float16 (same 1 col/cycle PE rate as f32r, but
FWL halves LDWEIGHTS and DMA volume halves); PSUM/gates/c-state stay
fp32. Dense-head weights are host-pre-shuffled to SBUF layout for
contiguous flat DMA.
Conv layers are computed as tap-accumulated matmuls in PSUM:
  - hidden (SAME) convs read shifted views of a zero-padded SBUF h state
  - input (VALID) convs are fused into the same PSUM accumulation group
  - layer-1's input conv uses host-side im2col (K=125)
  - 64-channel contractions pack 2 taps per matmul via a partition-
    duplicated, x-shifted copy (K=64 -> 128)
"""
import os
import sys
import types
from contextlib import ExitStack

import numpy as np

import concourse.bacc as bacc
import concourse.bass as bass
import concourse.mybir as mybir
import concourse.tile as tile
from concourse.bass_utils import run_bass_kernel_spmd

F32 = mybir.dt.float32
F32R = mybir.dt.float32r
F16 = mybir.dt.float16
AF = mybir.ActivationFunctionType
ALU = mybir.AluOpType

NCORES = 8
BC = 2          # batch per core
T = 6

# layer geometry
L1 = dict(Ho=60, Wo=60, F=128, NCT=4, Wp=64, Cin=128)
L2 = dict(Ho=56, Wo=56, F=64, NCT=2, Wp=60, Cin=128)
L3 = dict(Ho=52, Wo=52, F=64, NCT=2, Wp=56, Cin=64)

LAST_EXEC_NS = []
LAST_RESULTS = []

_CACHE = {}


def _want_trace():
    if os.environ.get("BASS_KERNEL_TRACE") != "1":
        return False
    try:
        _install_ntff_hook()
        return True
    except Exception:
        return False


def _install_ntff_hook():
    if "antenv.axon_hooks" in sys.modules:
        return
    mod = types.ModuleType("antenv.axon_hooks")
    mod._hook = None
    mod.set_axon_ntff_profile_hook = lambda h: setattr(mod, "_hook", h)
    mod.get_axon_ntff_profile_hook = lambda: mod._hook
    sys.modules["antenv.axon_hooks"] = mod
    import antenv
    antenv.axon_hooks = mod
    from trn_agent_boot.trn_boot import _ntff_profile_via_ctypes
    hook = _ntff_profile_via_ctypes("/opt/axon/libaxon_pjrt.so")
    if hook is not None:
        mod.set_axon_ntff_profile_hook(hook)


def _tap_view(t_ap, off, nrow, wp, wo):
    """[128, nrow, wo] strided view at free-dim offset `off`, row stride wp."""
    return t_ap[:, off:off + nrow * wp].rearrange("p (r w) -> p r w", r=nrow)[:, :, :wo]


def _blocks(ho, nrow):
    """Split ho rows into blocks of nrow (last blocks adjusted to keep N>=256)."""
    if ho == 60:                # L1: 6x8 + 2x6 (min N = 6*60=360)
        return [(i * 8, 8) for i in range(6)] + [(48, 6), (54, 6)]
    if ho == 56:                # L2: 7x8 (N = 448)
        return [(i * 8, 8) for i in range(7)]
    if ho == 52:                # L3: 5x9 + 1x7 (N = 468 / 364)
        return [(i * 9, 9) for i in range(5)] + [(45, 7)]
    raise ValueError(ho)


def _build_launch_a(debug=False):
    nc = bacc.Bacc("TRN2", target_bir_lowering=False, debug=False,
                   num_devices=NCORES)

    xim = nc.dram_tensor("xim", [BC * T, 125, 3600], F16, kind="ExternalInput").ap()
    wx1 = nc.dram_tensor("wx1", [125, 512], F16, kind="ExternalInput").ap()
    wh1 = nc.dram_tensor("wh1", [128, 25 * 512], F16, kind="ExternalInput").ap()
    wx2 = nc.dram_tensor("wx2", [128, 25 * 256], F16, kind="ExternalInput").ap()
    wh2p = nc.dram_tensor("wh2p", [128, 15 * 256], F16, kind="ExternalInput").ap()
    wx3p = nc.dram_tensor("wx3p", [128, 15 * 256], F16, kind="ExternalInput").ap()
    wh3p = nc.dram_tensor("wh3p", [128, 15 * 256], F16, kind="ExternalInput").ap()
    b1v = nc.dram_tensor("b1v", [128, 4], F32, kind="ExternalInput").ap()
    b2v = nc.dram_tensor("b2v", [64, 4], F32, kind="ExternalInput").ap()
    b3v = nc.dram_tensor("b3v", [64, 4], F32, kind="ExternalInput").ap()
    h3o = nc.dram_tensor("h3o", [128, 2704], F16, kind="ExternalOutput").ap()
    if debug:
        h1dbg = nc.dram_tensor("h1dbg", [BC * T, 128, 3600], F16, kind="ExternalOutput").ap()
        h2dbg = nc.dram_tensor("h2dbg", [BC * T, 64, 3136], F16, kind="ExternalOutput").ap()

    with TileCtx(nc) as tc, ExitStack() as top:
        dram = top.enter_context(tc.tile_pool(name="dram", bufs=1, space="DRAM"))
        if debug:
            h1seq, h2seq = h1dbg, h2dbg
        else:
            h1seq = dram.tile([BC * T, 128, 3600], F16)
            h2seq = dram.tile([BC * T, 64, 3136], F16)

        # ---------------- phase 1: ConvLSTM(5 -> 128), 60x60 ----------------
        with ExitStack() as ctx:
            wpool = ctx.enter_context(tc.tile_pool(name="w1", bufs=1))
            spool = ctx.enter_context(tc.tile_pool(name="s1", bufs=1))
            ipool = ctx.enter_context(tc.tile_pool(name="i1", bufs=6))
            gpool = ctx.enter_context(tc.tile_pool(name="g1", bufs=2))
            ppool = ctx.enter_context(tc.tile_pool(name="p1", bufs=8, space="PSUM"))

            wx1_t = wpool.tile([125, 512], F16, tag="wx1")
            wh1_t = wpool.tile([128, 25 * 512], F16, tag="wh1")
            nc.gpsimd.dma_start(out=wx1_t[:, :], in_=wx1[:, :])
            nc.gpsimd.dma_start(out=wh1_t[:, :], in_=wh1[:, :])
            bsb = wpool.tile([128, 4], F32, tag="b1raw")
            bsig = wpool.tile([128, 4], F32, tag="b1sig")
            nc.sync.dma_start(out=bsb[:, :], in_=b1v[:, :])
            nc.vector.tensor_scalar(bsig[:, :], bsb[:, :], 0.2, 0.5, ALU.mult, ALU.add)

            hpad = [spool.tile([128, 4160], F16, tag=f"hpad{b}", name=f"hpad1_{b}") for b in range(BC)]
            hcur = [spool.tile([128, 3600], F16, tag=f"hcur{b}", name=f"hcur1_{b}") for b in range(BC)]
            cst = [spool.tile([128, 3600], F32, tag=f"c1_{b}", name=f"c1_{b}") for b in range(BC)]
            for b in range(BC):
                nc.gpsimd.memset(hpad[b][:, :].bitcast(F32), 0.0)

            wp, wo = L1["Wp"], L1["Wo"]
            for t in range(T):
                for b in range(BC):
                    cts = (0, 2, 3) if t == 0 else (0, 1, 2, 3)
                    for y0, nrow in _blocks(60, 8):
                        n = nrow * wo
                        xb = ipool.tile([125, 480], F16, tag="xim")
                        nc.sync.dma_start(out=xb[:, :n],
                                          in_=xim[b * T + t, :, y0 * 60:y0 * 60 + n])
                        ps = {}
                        for ct in cts:
                            acc = ppool.tile([128, 480], F32, tag="ps")
                            ps[ct] = acc
                            nc.tensor.matmul(
                                acc[:, :n], wx1_t[:, ct * 128:(ct + 1) * 128],
                                xb[:, :n],
                                start=True, stop=(t == 0))
                            if t > 0:
                                for tap in range(25):
                                    dy, dx = divmod(tap, 5)
                                    rhs = _tap_view(hpad[b], (y0 + dy) * wp + dx, nrow, wp, wo)
                                    nc.tensor.matmul(
                                        acc[:, :n],
                                        wh1_t[:, tap * 512 + ct * 128:tap * 512 + (ct + 1) * 128],
                                        rhs, start=False, stop=(tap == 24))
                        sl = slice(y0 * wo, y0 * wo + n)
                        g = {}
                        for ct in cts:
                            gt = gpool.tile([128, 480], F32, tag=f"g{ct}")
                            g[ct] = gt
                            if ct == 2:
                                nc.scalar.activation(gt[:, :n], ps[ct][:, :n], AF.Tanh,
                                                     bias=bsb[:, 2:3])
                            else:
                                nc.scalar.activation(gt[:, :n], ps[ct][:, :n], AF.Identity,
                                                     bias=bsig[:, ct:ct + 1], scale=0.2)
                                nc.vector.tensor_scalar(gt[:, :n], gt[:, :n], 0.0, 1.0,
                                                        ALU.max, ALU.min)
                        if t == 0:
                            nc.vector.tensor_mul(cst[b][:, sl], g[0][:, :n], g[2][:, :n])
                        else:
                            t1 = gpool.tile([128, 480], F32, tag="t1")
                            t2 = gpool.tile([128, 480], F32, tag="t2")
                            nc.vector.tensor_mul(t1[:, :n], g[1][:, :n], cst[b][:, sl])
                            nc.vector.tensor_mul(t2[:, :n], g[0][:, :n], g[2][:, :n])
                            nc.vector.tensor_add(cst[b][:, sl], t1[:, :n], t2[:, :n])
                        tc_t = gpool.tile([128, 480], F32, tag="tct")
                        nc.scalar.activation(tc_t[:, :n], cst[b][:, sl], AF.Tanh)
                        nc.vector.tensor_mul(hcur[b][:, sl], g[3][:, :n], tc_t[:, :n])
                    # end blocks: update padded state + spill sequence
                    dst = _tap_view(hpad[b], 2 * wp + 2, wo, wp, wo)
                    src = hcur[b][:, :].rearrange("p (r w) -> p r w", r=wo)
                    nc.vector.tensor_copy(dst, src)
                    nc.sync.dma_start(out=h1seq[b * T + t, :, :], in_=hcur[b][:, :])

        # ---------------- phase 2: ConvLSTM(128 -> 64), 56x56 ----------------
        with ExitStack() as ctx:
            wpool = ctx.enter_context(tc.tile_pool(name="w2", bufs=1))
            spool = ctx.enter_context(tc.tile_pool(name="s2", bufs=1))
            ipool = ctx.enter_context(tc.tile_pool(name="i2", bufs=2))
            gpool = ctx.enter_context(tc.tile_pool(name="g2", bufs=3))
            ppool = ctx.enter_context(tc.tile_pool(name="p2", bufs=8, space="PSUM"))

            wx2_t = wpool.tile([128, 25 * 256], F16, tag="wx2")
            wh2_t = wpool.tile([128, 15 * 256], F16, tag="wh2")
            nc.gpsimd.dma_start(out=wx2_t[:, :], in_=wx2[:, :])
            nc.gpsimd.dma_start(out=wh2_t[:, :], in_=wh2p[:, :])
            bsb = wpool.tile([64, 4], F32, tag="b2raw")
            bsig = wpool.tile([64, 4], F32, tag="b2sig")
            nc.sync.dma_start(out=bsb[:, :], in_=b2v[:, :])
            nc.vector.tensor_scalar(bsig[:, :], bsb[:, :], 0.2, 0.5, ALU.mult, ALU.add)

            hpad = [spool.tile([128, 3664], F16, tag=f"hpad{b}", name=f"hpad2_{b}") for b in range(BC)]
            hcur = [spool.tile([64, 3136], F16, tag=f"hcur{b}", name=f"hcur2_{b}") for b in range(BC)]
            cst = [spool.tile([64, 3136], F32, tag=f"c2_{b}", name=f"c2_{b}") for b in range(BC)]
            for b in range(BC):
                nc.gpsimd.memset(hpad[b][:, :].bitcast(F32), 0.0)

            wp, wo = L2["Wp"], L2["Wo"]
            for t in range(T):
                for b in range(BC):
                    img = ipool.tile([128, 3640], F16, tag="h1in")
                    nc.sync.dma_start(out=img[:, :3600], in_=h1seq[b * T + t, :, :])
                    for y0, nrow in _blocks(56, 8):
                        n = nrow * wo
                        ps = []
                        for ct in range(2):
                            acc = ppool.tile([128, 448], F32, tag="ps")
                            ps.append(acc)
                            first = True
                            for tap in range(25):
                                dy, dx = divmod(tap, 5)
                                rhs = _tap_view(img, (y0 + dy) * 60 + dx, nrow, 60, wo)
                                nc.tensor.matmul(
                                    acc[:, :n],
                                    wx2_t[:, tap * 256 + ct * 128:tap * 256 + (ct + 1) * 128],
                                    rhs, start=first,
                                    stop=(t == 0 and tap == 24))
                                first = False
                            if t > 0:
                                for e in range(15):
                                    dy, k = divmod(e, 3)
                                    rhs = _tap_view(hpad[b], (y0 + dy) * wp + 2 * k, nrow, wp, wo)
                                    nc.tensor.matmul(
                                        acc[:, :n],
                                        wh2_t[:, e * 256 + ct * 128:e * 256 + (ct + 1) * 128],
                                        rhs, start=False, stop=(e == 14))
                        sl = slice(y0 * wo, y0 * wo + n)
                        # gates: ps[0]=[i;f], ps[1]=[c;o]
                        si = gpool.tile([64, 448], F32, tag="si")
                        nc.scalar.activation(si[:, :n], ps[0][0:64, :n], AF.Identity,
                                             bias=bsig[:, 0:1], scale=0.2)
                        nc.vector.tensor_scalar(si[:, :n], si[:, :n], 0.0, 1.0,
                                                ALU.max, ALU.min)
                        gt = gpool.tile([64, 448], F32, tag="gt")
                        nc.scalar.activation(gt[:, :n], ps[1][0:64, :n], AF.Tanh,
                                             bias=bsb[:, 2:3])
                        so = gpool.tile([64, 448], F32, tag="so")
                        nc.scalar.activation(so[:, :n], ps[1][64:128, :n], AF.Identity,
                                             bias=bsig[:, 3:4], scale=0.2)
                        nc.vector.tensor_scalar(so[:, :n], so[:, :n], 0.0, 1.0,
                                                ALU.max, ALU.min)
                        if t == 0:
                            nc.vector.tensor_mul(cst[b][:, sl], si[:, :n], gt[:, :n])
                        else:
                            sf = gpool.tile([64, 448], F32, tag="sf")
                            nc.scalar.activation(sf[:, :n], ps[0][64:128, :n], AF.Identity,
                                                 bias=bsig[:, 1:2], scale=0.2)
                            nc.vector.tensor_scalar(sf[:, :n], sf[:, :n], 0.0, 1.0,
                                                    ALU.max, ALU.min)
                            t1 = gpool.tile([64, 448], F32, tag="t1")
                            t2 = gpool.tile([64, 448], F32, tag="t2")
                            nc.vector.tensor_mul(t1[:, :n], sf[:, :n], cst[b][:, sl])
                            nc.vector.tensor_mul(t2[:, :n], si[:, :n], gt[:, :n])
                            nc.vector.tensor_add(cst[b][:, sl], t1[:, :n], t2[:, :n])
                        tc_t = gpool.tile([64, 448], F32, tag="tct")
                        nc.scalar.activation(tc_t[:, :n], cst[b][:, sl], AF.Tanh)
                        nc.vector.tensor_mul(hcur[b][:, sl], so[:, :n], tc_t[:, :n])
                    # end blocks: padded dup state (rows 0:64 plain, 64:128 x-shifted)
                    src = hcur[b][:, :].rearrange("p (r w) -> p r w", r=wo)
                    dst0 = hpad[b][0:64, 2 * wp + 2:2 * wp + 2 + wo * wp] \
                        .rearrange("p (r w) -> p r w", r=wo)[:, :, :wo]
                    nc.vector.tensor_copy(dst0, src)
                    dst1 = hpad[b][64:128, 2 * wp + 1:2 * wp + 1 + wo * wp] \
                        .rearrange("p (r w) -> p r w", r=wo)[:, :, :wo]
                    nc.vector.tensor_copy(dst1, src)
                    nc.sync.dma_start(out=h2seq[b * T + t, :, :], in_=hcur[b][:, :])

        # ---------------- phase 3: ConvLSTM(64 -> 64), 52x52 ----------------
        with ExitStack() as ctx:
            wpool = ctx.enter_context(tc.tile_pool(name="w3", bufs=1))
            spool = ctx.enter_context(tc.tile_pool(name="s3", bufs=1))
            ipool = ctx.enter_context(tc.tile_pool(name="i3", bufs=2))
            gpool = ctx.enter_context(tc.tile_pool(name="g3", bufs=3))
            ppool = ctx.enter_context(tc.tile_pool(name="p3", bufs=8, space="PSUM"))

            wx3_t = wpool.tile([128, 15 * 256], F16, tag="wx3")
            wh3_t = wpool.tile([128, 15 * 256], F16, tag="wh3")
            nc.gpsimd.dma_start(out=wx3_t[:, :], in_=wx3p[:, :])
            nc.gpsimd.dma_start(out=wh3_t[:, :], in_=wh3p[:, :])
            bsb = wpool.tile([64, 4], F32, tag="b3raw")
            bsig = wpool.tile([64, 4], F32, tag="b3sig")
            nc.sync.dma_start(out=bsb[:, :], in_=b3v[:, :])
            nc.vector.tensor_scalar(bsig[:, :], bsb[:, :], 0.2, 0.5, ALU.mult, ALU.add)

            hpad = [spool.tile([128, 3300], F16, tag=f"hpad{b}", name=f"hpad3_{b}") for b in range(BC)]
            hcur = [spool.tile([64, 2704], F16, tag=f"hcur{b}", name=f"hcur3_{b}") for b in range(BC)]
            cst = [spool.tile([64, 2704], F32, tag=f"c3_{b}", name=f"c3_{b}") for b in range(BC)]
            for b in range(BC):
                nc.gpsimd.memset(hpad[b][:, :].bitcast(F32), 0.0)

            wp, wo = L3["Wp"], L3["Wo"]
            for t in range(T):
                for b in range(BC):
                    # build dup input [128, 3196]: rows 0:64 = h2 image,
                    # rows 64:128 = x-shifted by 1 (valid 56-wide coords)
                    img = ipool.tile([128, 3196], F16, tag="h2in")
                    nc.sync.dma_start(out=img[0:64, :3136], in_=h2seq[b * T + t, :, :])
                    nc.vector.tensor_copy(img[64:128, 0:3135], img[0:64, 1:3136])
                    for y0, nrow in _blocks(52, 9):
                        n = nrow * wo
                        ps = []
                        for ct in range(2):
                            acc = ppool.tile([128, 468], F32, tag="ps")
                            ps.append(acc)
                            first = True
                            for e in range(15):
                                dy, k = divmod(e, 3)
                                rhs = _tap_view(img, (y0 + dy) * 56 + 2 * k, nrow, 56, wo)
                                nc.tensor.matmul(
                                    acc[:, :n],
                                    wx3_t[:, e * 256 + ct * 128:e * 256 + (ct + 1) * 128],
                                    rhs, start=first,
                                    stop=(t == 0 and e == 14))
                                first = False
                            if t > 0:
                                for e in range(15):
                                    dy, k = divmod(e, 3)
                                    rhs = _tap_view(hpad[b], (y0 + dy) * wp + 2 * k, nrow, wp, wo)
                                    nc.tensor.matmul(
                                        acc[:, :n],
                                        wh3_t[:, e * 256 + ct * 128:e * 256 + (ct + 1) * 128],
                                        rhs, start=False, stop=(e == 14))
                        sl = slice(y0 * wo, y0 * wo + n)
                        si = gpool.tile([64, 468], F32, tag="si")
                        nc.scalar.activation(si[:, :n], ps[0][0:64, :n], AF.Identity,
                                             bias=bsig[:, 0:1], scale=0.2)
                        nc.vector.tensor_scalar(si[:, :n], si[:, :n], 0.0, 1.0,
                                                ALU.max, ALU.min)
                        gt = gpool.tile([64, 468], F32, tag="gt")
                        nc.scalar.activation(gt[:, :n], ps[1][0:64, :n], AF.Tanh,
                                             bias=bsb[:, 2:3])
                        so = gpool.tile([64, 468], F32, tag="so")
                        nc.scalar.activation(so[:, :n], ps[1][64:128, :n], AF.Identity,
                                             bias=bsig[:, 3:4], scale=0.2)
                        nc.vector.tensor_scalar(so[:, :n], so[:, :n], 0.0, 1.0,
                                                ALU.max, ALU.min)
                        if t == 0:
                            nc.vector.tensor_mul(cst[b][:, sl], si[:, :n], gt[:, :n])
                        else:
                            sf = gpool.tile([64, 468], F32, tag="sf")
                            nc.scalar.activation(sf[:, :n], ps[0][64:128, :n], AF.Identity,
                                                 bias=bsig[:, 1:2], scale=0.2)
                            nc.vector.tensor_scalar(sf[:, :n], sf[:, :n], 0.0, 1.0,
                                                    ALU.max, ALU.min)
                            t1 = gpool.tile([64, 468], F32, tag="t1")
                            t2 = gpool.tile([64, 468], F32, tag="t2")
                            nc.vector.tensor_mul(t1[:, :n], sf[:, :n], cst[b][:, sl])
                            nc.vector.tensor_mul(t2[:, :n], si[:, :n], gt[:, :n])
                            nc.vector.tensor_add(cst[b][:, sl], t1[:, :n], t2[:, :n])
                        tc_t = gpool.tile([64, 468], F32, tag="tct")
                        nc.scalar.activation(tc_t[:, :n], cst[b][:, sl], AF.Tanh)
                        nc.vector.tensor_mul(hcur[b][:, sl], so[:, :n], tc_t[:, :n])
                    if t < T - 1:
                        src = hcur[b][:, :].rearrange("p (r w) -> p r w", r=wo)
                        dst0 = hpad[b][0:64, 2 * wp + 2:2 * wp + 2 + wo * wp] \
                            .rearrange("p (r w) -> p r w", r=wo)[:, :, :wo]
                        nc.vector.tensor_copy(dst0, src)
                        dst1 = hpad[b][64:128, 2 * wp + 1:2 * wp + 1 + wo * wp] \
                            .rearrange("p (r w) -> p r w", r=wo)[:, :, :wo]
                        nc.vector.tensor_copy(dst1, src)
                    else:
                        nc.sync.dma_start(out=h3o[b * 64:(b + 1) * 64, :], in_=hcur[b][:, :])

    nc.compile()
    return nc


def TileCtx(nc):
    return tile.TileContext(nc, pool_alloc_mode="queue")


def _build_launch_b():
    KS = 173056 // NCORES      # 21632 contraction rows per core
    KT = KS // 128             # 169 k-tiles
    CH = 8                     # k-tiles per DMA chunk
    nc = bacc.Bacc("TRN2", target_bir_lowering=False, debug=False,
                   num_devices=NCORES)
    ztk = nc.dram_tensor("ztk", [128, (KS // 128) * 16], F16, kind="ExternalInput").ap()
    # wd1 halves pre-shuffled on host to SBUF layout [p, kt*512] for flat DMA
    wd1a = nc.dram_tensor("wd1a", [128, KT * 512], F16, kind="ExternalInput").ap()
    wd1b = nc.dram_tensor("wd1b", [128, KT * 512], F16, kind="ExternalInput").ap()
    wd2 = nc.dram_tensor("wd2", [128, 8 * 1024], F16, kind="ExternalInput").ap()
    wd3 = nc.dram_tensor("wd3", [128, 8 * 4], F16, kind="ExternalInput").ap()
    bd1 = nc.dram_tensor("bd1", [128, 8], F32, kind="ExternalInput").ap()
    bd2 = nc.dram_tensor("bd2", [128, 8], F32, kind="ExternalInput").ap()
    bd3 = nc.dram_tensor("bd3", [4, 1], F32, kind="ExternalInput").ap()
    eye = nc.dram_tensor("eye16", [16, 16], F32, kind="ExternalInput").ap()
    out = nc.dram_tensor("out", [4, 16], F32, kind="ExternalOutput").ap()

    with TileCtx(nc) as tc, ExitStack() as ctx:
        cpool = ctx.enter_context(tc.tile_pool(name="cst", bufs=1))
        wpool = ctx.enter_context(tc.tile_pool(name="wd1", bufs=3))
        apool = ctx.enter_context(tc.tile_pool(name="act", bufs=1))
        ppool = ctx.enter_context(tc.tile_pool(name="ps", bufs=1, space="PSUM"))
        dram = ctx.enter_context(tc.tile_pool(name="dram", bufs=1, space="DRAM"))

        # z^T slice: [KS,16] -> [128, KT*16]
        zt = cpool.tile([128, KT * 16], F16, tag="zt")
        nc.gpsimd.dma_start(out=zt[:, :], in_=ztk[:, :])
        eye_t = cpool.tile([16, 16], F32, tag="eye")
        nc.sync.dma_start(out=eye_t[:, :], in_=eye[:, :])
        b1t = cpool.tile([128, 8], F32, tag="b1")
        b2t = cpool.tile([128, 8], F32, tag="b2")
        b3t = cpool.tile([4, 1], F32, tag="b3")
        nc.sync.dma_start(out=b1t[:, :], in_=bd1[:, :])
        nc.sync.dma_start(out=b2t[:, :], in_=bd2[:, :])
        nc.sync.dma_start(out=b3t[:, :], in_=bd3[:, :])
        wd2_t = cpool.tile([128, 8 * 1024], F16, tag="wd2")
        nc.gpsimd.dma_start(out=wd2_t[:, :], in_=wd2[:, :])
        wd3_t = cpool.tile([128, 8 * 4], F16, tag="wd3")
        nc.gpsimd.dma_start(out=wd3_t[:, :], in_=wd3[:, :])

        # dense1 in two column halves; half 0's AllReduce overlaps half 1
        bin_ = [dram.tile([16, 512], F32, name=f"bin{h}") for h in range(2)]
        bout = [dram.tile([16, 512], F32, name=f"bout{h}") for h in range(2)]
        chunks = []
        _off = 0
        for _sz in [2, 2, 4]:
            chunks.append((_off, _sz))
            _off += _sz
        while _off < KT:
            _sz = min(CH, KT - _off)
            chunks.append((_off, _sz))
            _off += _sz
        for h, src in ((0, wd1a), (1, wd1b)):
            acc = ppool.tile([16, 512], F32, tag="acc", name=f"acc{h}", bufs=2)
            for c0, cn in chunks:
                w_t = wpool.tile([128, CH * 512], F16, tag="w", name=f"w{h}_{c0}")
                nc.sync.dma_start(out=w_t[:, :cn * 512],
                                  in_=src[:, c0 * 512:(c0 + cn) * 512])
                for i in range(cn):
                    kt = c0 + i
                    nc.tensor.matmul(acc[:, :], zt[:, kt * 16:(kt + 1) * 16],
                                     w_t[:, i * 512:(i + 1) * 512],
                                     start=(kt == 0), stop=(kt == KT - 1))
            a1p = apool.tile([16, 512], F32, tag="a1p", name=f"a1p{h}", bufs=2)
            nc.vector.tensor_copy(a1p[:, :], acc[:, :])
            nc.sync.dma_start(out=bin_[h][:, :], in_=a1p[:, :])
            nc.gpsimd.collective_compute(
                "AllReduce", ALU.add,
                replica_groups=[list(range(NCORES))],
                ins=[bin_[h][:].opt()], outs=[bout[h][:].opt()])
        a1f = apool.tile([16, 1024], F32, tag="a1f")
        for h in range(2):
            nc.sync.dma_start(out=a1f[:, h * 512:(h + 1) * 512], in_=bout[h][:, :])

        # transpose a1 -> [128,16] tiles; bias+relu; dense2
        a1t = apool.tile([128, 8 * 16], F16, tag="a1t")
        for ct in range(8):
            pt = ppool.tile([128, 16], F32, tag="pt", bufs=2)
            nc.tensor.transpose(pt[:, :], a1f[:, ct * 128:(ct + 1) * 128],
                                eye_t[:, :])
            nc.scalar.activation(a1t[:, ct * 16:(ct + 1) * 16], pt[:, :], AF.Relu,
                                 bias=b1t[:, ct:ct + 1])
        a2t = apool.tile([128, 8 * 16], F16, tag="a2t")
        for ct in range(8):
            p2 = ppool.tile([128, 16], F32, tag="p2", bufs=2)
            for kt in range(8):
                nc.tensor.matmul(
                    p2[:, :],
                    wd2_t[:, kt * 1024 + ct * 128:kt * 1024 + (ct + 1) * 128],
                    a1t[:, kt * 16:(kt + 1) * 16],
                    start=(kt == 0), stop=(kt == 7))
            nc.scalar.activation(a2t[:, ct * 16:(ct + 1) * 16], p2[:, :], AF.Relu,
                                 bias=b2t[:, ct:ct + 1])
        p3 = ppool.tile([4, 16], F32, tag="p3", bufs=1)
        for kt in range(8):
            nc.tensor.matmul(p3[:, :], wd3_t[:, kt * 4:(kt + 1) * 4],
                             a2t[:, kt * 16:(kt + 1) * 16],
                             start=(kt == 0), stop=(kt == 7))
        o_t = apool.tile([4, 16], F32, tag="o")
        nc.vector.tensor_scalar(o_t[:, :], p3[:, :], b3t[:, 0:1], None, ALU.add)
        nc.sync.dma_start(out=out[:, :], in_=o_t[:, :])

    nc.compile()
    return nc


def _pack_pairs(w):
    """(5,5,64,256) -> [128, 15*256]: pair taps (dy,2k)+(dy,2k+1) along K."""
    out = np.zeros((128, 15, 256), np.float32)
    for dy in range(5):
        for k in range(3):
            e = dy * 3 + k
            out[0:64, e] = w[dy, 2 * k]
            if 2 * k + 1 < 5:
                out[64:128, e] = w[dy, 2 * k + 1]
    return np.ascontiguousarray(out.reshape(128, 15 * 256)).astype(np.float16)


def _host_prep_a(x, Wx1, Wh1, b1, Wx2, Wh2, b2, Wx3, Wh3, b3):
    xw = np.lib.stride_tricks.sliding_window_view(x, (5, 5), axis=(2, 3))
    # [b,t,y,x,c,dy,dx] -> [b,t,(dy,dx,c),(y,x)]
    xim = np.ascontiguousarray(
        xw.transpose(0, 1, 5, 6, 4, 2, 3).reshape(16, 6, 125, 3600), np.float32)
    shared = dict(
        wx1=np.ascontiguousarray(Wx1.reshape(125, 512), np.float32).astype(np.float16),
        wh1=np.ascontiguousarray(
            Wh1.reshape(25, 128, 512).transpose(1, 0, 2).reshape(128, 25 * 512)).astype(np.float16),
        wx2=np.ascontiguousarray(
            Wx2.reshape(25, 128, 256).transpose(1, 0, 2).reshape(128, 25 * 256)).astype(np.float16),
        wh2p=_pack_pairs(Wh2.reshape(5, 5, 64, 256)),
        wx3p=_pack_pairs(Wx3.reshape(5, 5, 64, 256)),
        wh3p=_pack_pairs(Wh3.reshape(5, 5, 64, 256)),
        b1v=np.ascontiguousarray(b1.reshape(4, 128).T, np.float32),
        b2v=np.ascontiguousarray(b2.reshape(4, 64).T, np.float32),
        b3v=np.ascontiguousarray(b3.reshape(4, 64).T, np.float32),
    )
    in_maps = []
    for j in range(NCORES):
        m = dict(shared)
        m["xim"] = np.ascontiguousarray(
            xim[2 * j:2 * j + 2].reshape(12, 125, 3600)).astype(np.float16)
        in_maps.append(m)
    return in_maps


def _run(nc, in_maps, trace):
    res = run_bass_kernel_spmd(nc, in_maps, core_ids=list(range(NCORES)),
                               trace=trace)
    if res.exec_time_ns is not None:
        LAST_EXEC_NS.append(res.exec_time_ns)
    LAST_RESULTS.append(res)
    return res


def kernel(x, Wx1, Wh1, b1, Wx2, Wh2, b2, Wx3, Wh3, b3,
           Wd1, bd1, Wd2, bd2, Wd3, bd3):
    trace = _want_trace()
    LAST_EXEC_NS.clear()
    LAST_RESULTS.clear()
    x = np.asarray(x, np.float32)

    if "a" not in _CACHE:
        _CACHE["a"] = _build_launch_a()
    in_a = _host_prep_a(x, np.asarray(Wx1), np.asarray(Wh1), np.asarray(b1),
                        np.asarray(Wx2), np.asarray(Wh2), np.asarray(b2),
                        np.asarray(Wx3), np.asarray(Wh3), np.asarray(b3))
    res_a = _run(_CACHE["a"], in_a, trace)

    h3 = np.stack([res_a.results[j]["h3o"][(b % 2) * 64:(b % 2) * 64 + 64]
                   for b, j in [(b, b // 2) for b in range(16)]]).astype(np.float32)
    zt = np.ascontiguousarray(h3.transpose(2, 1, 0).reshape(173056, 16), np.float32)

    if "b" not in _CACHE:
        _CACHE["b"] = _build_launch_b()
    KS = 173056 // NCORES
    KT = KS // 128
    Wd1 = np.asarray(Wd1, np.float32)
    shared_b = dict(
        wd2=np.ascontiguousarray(
            np.asarray(Wd2, np.float32).reshape(8, 128, 1024)
            .transpose(1, 0, 2).reshape(128, 8 * 1024)).astype(np.float16),
        wd3=np.ascontiguousarray(
            np.asarray(Wd3, np.float32).reshape(8, 128, 4)
            .transpose(1, 0, 2).reshape(128, 32)).astype(np.float16),
        bd1=np.ascontiguousarray(np.asarray(bd1).reshape(8, 128).T, np.float32),
        bd2=np.ascontiguousarray(np.asarray(bd2).reshape(8, 128).T, np.float32),
        bd3=np.asarray(bd3, np.float32).reshape(4, 1),
        eye16=np.eye(16, dtype=np.float32),
    )
    in_b = []
    for j in range(NCORES):
        m = dict(shared_b)
        zs = zt[j * KS:(j + 1) * KS]                    # [KS, 16]
        m["ztk"] = np.ascontiguousarray(
            zs.reshape(KT, 128, 16).transpose(1, 0, 2).reshape(128, -1)).astype(np.float16)
        for key, h in (("wd1a", 0), ("wd1b", 1)):
            m[key] = np.ascontiguousarray(
                Wd1[j * KS:(j + 1) * KS, h * 512:(h + 1) * 512]
                .reshape(KT, 128, 512).transpose(1, 0, 2)
                .reshape(128, KT * 512)).astype(np.float16)
        in_b.append(m)
    res_b = _run(_CACHE["b"], in_b, trace)

    return np.ascontiguousarray(res_b.results[0]["out"].T, np.float32)

